# revision 16
# baseline (speedup 1.0000x reference)
"""Trainium2 Bass kernel for nn_CrossTransFormer (grouped-proj cross attention
with Gram-matrix sparsity loss).

Reference computation (per batch b; C=1024, G=4 groups, Pk=1024, Pq=4096):
    Q = blockdiag(Wq) @ Xk + bq          [C, Pk]
    K = blockdiag(Wk) @ Xk + bk          [C, Pk]
    V = blockdiag(Wv) @ Xq + bv          [C, Pq]
    Hraw[i,j] = sum_c K[c,i] V[c,j] * SCALE       [Pk, Pq]
    H = softmax(Hraw, axis=i)
    mid = H @ H.T                        [Pk, Pk]
    sparse_loss = mean((mid - I)^2)      (over all batches)
    out = Q @ H + 0                      [C, Pq]   (out[k,j] = sum_i Q[k,i] H[i,j])

Sharding: data-parallel over batch, one batch per NeuronCore (8 cores).

Per-core dataflow (all matmuls bf16 with fp32 PSUM accumulation):
  - K  [C part, Pk]        k[co,p] = Wk-proj(Xk) + bk   (bias per-partition)
  - QT [Pk part, C]        qt[p,co] = Q0^T (NO bias: out = Q0@H + bq since
                           softmax columns sum to 1 -> bq added at out evac)
  - V  [C part, Pq-block]  streamed per 512-col block of Pq
  - HT [Pq part, Pk]       ht[j,i] = exp(SCALE * sum_c V[c,j]K[c,i]) / S[j]
                           (exp via ScalarE with accum_out = row sum S[j];
                           softmax over i = free axis -> per-partition scalar)
  - ht -> HBM (bf16) is the H output (host transposes + casts)
  - Gram mid = HT^T-contraction over j (= partitions) directly from ht tiles;
    loss partials: sum(mid^2) per tile via ACT Square accum_out, diagonal via
    tensor_tensor_reduce against identity; host: sum(sq) - 2*sum(diag) + Pk.
  - out: H (untransposed) recovered via dma_start_transpose from the ht HBM
    tensor; out[k,j] accumulated over Pk tiles; bias bq per-partition at evac.
"""

import os
from contextlib import ExitStack

import ml_dtypes
import numpy as np

import concourse.bass as bass
import concourse.mybir as mybir
import concourse.tile as tile
from concourse import bacc
from concourse.bass_utils import run_bass_kernel_spmd
from concourse.masks import make_identity

B = 8
C = 1024
G = 4
CG = C // G  # 256
PK = 1024
PQ = 4096
SCALE = float(1.0 / np.float32(np.sqrt(np.float32(1024.0))))

BF16 = mybir.dt.bfloat16
F32 = mybir.dt.float32
F8 = mybir.dt.float8e4
# fp8 pre-scale for the Gram operand: softmax weights (~1e-3) sit below
# fp8e4m3's subnormal range; x64 recenters them (validated: loss rel err
# 4e-5 vs 1e-2 unscaled)
F8S = 64.0
AF = mybir.ActivationFunctionType
ALU = mybir.AluOpType

N_CT = C // 128   # 8 channel tiles
N_PT = PK // 128  # 8 pk tiles
N_JT = PQ // 128  # 32 pq tiles
N_JB = PQ // 512  # 8 pq blocks
N_IB = PK // 512  # 2 pk blocks

_CACHE: dict = {}


def _build_nc():
    nc = bacc.Bacc("TRN2", target_bir_lowering=False, debug=False)

    # ---- DRAM I/O (per-core slices) ----
    xq_d = nc.dram_tensor("xq", [C, PQ], BF16, kind="ExternalInput").ap()
    xk_d = nc.dram_tensor("xk", [C, PK], BF16, kind="ExternalInput").ap()
    wqt_d = nc.dram_tensor("wqt", [C, CG], BF16, kind="ExternalInput").ap()
    wkt_d = nc.dram_tensor("wkt", [C, CG], BF16, kind="ExternalInput").ap()
    wvt_d = nc.dram_tensor("wvt", [C, CG], BF16, kind="ExternalInput").ap()
    bq_d = nc.dram_tensor("bq", [C], F32, kind="ExternalInput").ap()
    bk_d = nc.dram_tensor("bk", [C], F32, kind="ExternalInput").ap()
    bv_d = nc.dram_tensor("bv", [C], F32, kind="ExternalInput").ap()

    out_d = nc.dram_tensor("out", [C, PQ], F32, kind="ExternalOutput").ap()
    ht_d = nc.dram_tensor("ht", [PQ, PK], BF16, kind="ExternalOutput").ap()
    # loss partials (Gram computed on upper stair only, mid is symmetric):
    #  cols 0:12  = sum(mid^2) per computed chunk
    #  cols 12:20 = sum(mid^2) over the 8 diagonal 128x128 blocks
    #  cols 20:28 = sum(diag(mid)) per pk-tile
    loss_d = nc.dram_tensor("loss", [128, 28], F32, kind="ExternalOutput").ap()

    with TileCtx(nc) as (tc, ctx):
        _emit(nc, tc, ctx, xq_d, xk_d, wqt_d, wkt_d, wvt_d, bq_d, bk_d, bv_d,
              out_d, ht_d, loss_d)
    nc.compile()
    return nc


class TileCtx:
    def __init__(self, nc):
        self.nc = nc

    def __enter__(self):
        self.ctx = ExitStack()
        self.tc = self.ctx.enter_context(tile.TileContext(self.nc))
        return self.tc, self.ctx

    def __exit__(self, *exc):
        return self.ctx.__exit__(*exc)


def _emit(nc, tc, ctx, xq_d, xk_d, wqt_d, wkt_d, wvt_d, bq_d, bk_d, bv_d,
          out_d, ht_d, loss_d):
    # ---- persistent pools ----
    consts = ctx.enter_context(tc.tile_pool(name="consts", bufs=1))
    qt_pool = ctx.enter_context(tc.tile_pool(name="qt", bufs=1))
    psum = ctx.enter_context(tc.tile_pool(name="psum", bufs=8, space="PSUM"))
    acc_pool = ctx.enter_context(tc.tile_pool(name="accs", bufs=1))

    # ---- constants: all on the sync ring, ordered so the first K-proj
    # matmul only waits for wkt + the first two xk tiles ----
    wqt_sb = consts.tile([128, N_CT, CG], BF16, name="wqt_sb")
    wkt_sb = consts.tile([128, N_CT, CG], BF16, name="wkt_sb")
    wvt_sb = consts.tile([128, N_CT, CG], BF16, name="wvt_sb")
    bq_sb = consts.tile([128, N_CT], F32, name="bq_sb")
    bk_sb = consts.tile([128, N_CT], F32, name="bk_sb")
    bv_sb = consts.tile([128, N_CT], F32, name="bv_sb")
    nc.sync.dma_start(out=wkt_sb, in_=wkt_d.rearrange("(t p) c -> p t c", p=128))
    eye_sb = consts.tile([128, 128], F32, name="eye_sb")
    make_identity(nc, eye_sb)

    loss_sb = acc_pool.tile([128, 28], F32, name="loss_sb")

    qt_sb = qt_pool.tile([128, N_PT, C], BF16, name="qt_sb")

    f8_pool = ctx.enter_context(tc.tile_pool(name="f8", bufs=1))
    # Gram operand: H^T normalized, pre-scaled by F8S, cast to fp8e4m3.
    ht_f8 = f8_pool.tile([128, N_JT, PK], F8, name="ht_f8")

    with tc.tile_pool(name="big", bufs=1) as big_pool:
        ht_sb = big_pool.tile([128, N_JT, PK], BF16, name="ht_sb")

        with tc.tile_pool(name="ph12", bufs=1) as ph12, \
             tc.tile_pool(name="stream", bufs=2) as stream, \
             tc.tile_pool(name="evac", bufs=3) as evac, \
             tc.tile_pool(name="small", bufs=6) as small:
            xk_sb = ph12.tile([128, N_CT, PK], BF16, name="xk_sb")
            k_sb = ph12.tile([128, N_CT, PK], BF16, name="k_sb")
            xk_r = xk_d.rearrange("(t p) c -> p t c", p=128)
            nc.sync.dma_start(out=xk_sb[:, 0:2], in_=xk_r[:, 0:2])
            nc.sync.dma_start(out=bk_sb,
                              in_=bk_d.rearrange("(t p) -> p t", p=128))
            nc.sync.dma_start(out=xk_sb[:, 2:4], in_=xk_r[:, 2:4])
            nc.sync.dma_start(out=wqt_sb,
                              in_=wqt_d.rearrange("(t p) c -> p t c", p=128))
            nc.sync.dma_start(out=xk_sb[:, 4:6], in_=xk_r[:, 4:6])
            nc.sync.dma_start(out=xk_sb[:, 6:8], in_=xk_r[:, 6:8])
            nc.sync.dma_start(out=wvt_sb,
                              in_=wvt_d.rearrange("(t p) c -> p t c", p=128))
            nc.sync.dma_start(out=bv_sb,
                              in_=bv_d.rearrange("(t p) -> p t", p=128))
            nc.sync.dma_start(out=bq_sb,
                              in_=bq_d.rearrange("(t p) -> p t", p=128))

            # ---- Phase 1a: K projection -> k_sb [co part, pk] + bias ----
            for ct in range(N_CT):
                g, h = ct // 2, ct % 2
                for pb in range(N_IB):
                    ps = psum.tile([128, 512], F32, tag="ps", name="ps_k")
                    for ci in range(2):
                        nc.tensor.matmul(
                            ps,
                            wkt_sb[:, 2 * g + ci, h * 128:(h + 1) * 128],
                            xk_sb[:, 2 * g + ci, pb * 512:(pb + 1) * 512],
                            start=(ci == 0), stop=(ci == 1))
                    nc.vector.tensor_scalar_add(
                        k_sb[:, ct, pb * 512:(pb + 1) * 512], ps,
                        bk_sb[:, ct:ct + 1])

            # ---- Phase 1b: QT projection -> qt_sb [pk part, co] (no bias) ----
            for pt in range(N_PT):
                for g in range(G):
                    ps = psum.tile([128, 512], F32, tag="ps", name="ps_qt")
                    for ci in range(2):
                        nc.tensor.matmul(
                            ps[:, 0:CG],
                            xk_sb[:, 2 * g + ci, pt * 128:(pt + 1) * 128],
                            wqt_sb[:, 2 * g + ci, :],
                            start=(ci == 0), stop=(ci == 1))
                    nc.vector.tensor_copy(
                        qt_sb[:, pt, g * CG:(g + 1) * CG], ps[:, 0:CG])

            # ---- Phase 2: V projection + HT matmul + softmax, per Pq block --
            for jb in range(N_JB):
                xq_blk = stream.tile([128, N_CT, 512], BF16, tag="xq",
                                     name="xq_blk")
                nc.sync.dma_start(
                    out=xq_blk,
                    in_=xq_d[:, jb * 512:(jb + 1) * 512]
                        .rearrange("(t p) c -> p t c", p=128))
                v_blk = stream.tile([128, N_CT, 512], BF16, tag="v",
                                    name="v_blk")
                for ct in range(N_CT):
                    g, h = ct // 2, ct % 2
                    ps = psum.tile([128, 512], F32, tag="ps", name="ps_v")
                    for ci in range(2):
                        nc.tensor.matmul(
                            ps,
                            wvt_sb[:, 2 * g + ci, h * 128:(h + 1) * 128],
                            xq_blk[:, 2 * g + ci, :],
                            start=(ci == 0), stop=(ci == 1))
                    nc.scalar.activation(
                        out=v_blk[:, ct, :], in_=ps, func=AF.Identity,
                        bias=bv_sb[:, ct:ct + 1])

                for jt4 in range(4):
                    jt = jb * 4 + jt4
                    sacc = small.tile([128, 2], F32, tag="sacc", name="sacc")
                    for ib in range(N_IB):
                        ps = psum.tile([128, 512], F32, tag="ps", name="ps_h")
                        for ct in range(N_CT):
                            nc.tensor.matmul(
                                ps,
                                v_blk[:, ct, jt4 * 128:(jt4 + 1) * 128],
                                k_sb[:, ct, ib * 512:(ib + 1) * 512],
                                start=(ct == 0), stop=(ct == N_CT - 1))
                        nc.scalar.activation(
                            out=ht_sb[:, jt, ib * 512:(ib + 1) * 512],
                            in_=ps, func=AF.Exp, scale=SCALE,
                            accum_out=sacc[:, ib:ib + 1])
                    s_t = small.tile([128, 1], F32, tag="s", name="s_t")
                    nc.vector.tensor_add(s_t, sacc[:, 0:1], sacc[:, 1:2])
                    rs_t = small.tile([128, 1], F32, tag="rs", name="rs_t")
                    nc.vector.reciprocal(rs_t, s_t)
                    nc.vector.tensor_scalar_mul(
                        ht_sb[:, jt, :], ht_sb[:, jt, :], rs_t)
                    nc.sync.dma_start(out=ht_d[jt * 128:(jt + 1) * 128, :],
                                      in_=ht_sb[:, jt, :])
                    nc.vector.tensor_scalar_mul(
                        ht_f8[:, jt, :], ht_sb[:, jt, :], F8S)

        # ---- Phases 3+4 (ph12/stream pools closed; hn coexists with ht) --
        with tc.tile_pool(name="hn", bufs=1) as hn_pool, \
             tc.tile_pool(name="gscr", bufs=3) as gscr_pool, \
             tc.tile_pool(name="ostage", bufs=3) as ostage:
            # transpose-load H from the ht HBM tensor; emitted before the
            # Gram matmuls so the DMAs overlap Gram compute on the PE.
            hn_sb = hn_pool.tile([128, N_PT, PQ], BF16, name="hn_sb")
            for it in range(N_PT):
                nc.sync.dma_start_transpose(
                    out=hn_sb[:, it, :], in_=ht_d[:, it * 128:(it + 1) * 128])

            # ---- Phase 3: Gram upper-stair chunks + loss partials ----
            # mid is symmetric: for pk-tile ct compute columns [ct*128, PK)
            # in chunks of <=512.  Host reconstructs the full-matrix sums.
            ci = 0
            for ct in range(N_PT):
                c0 = ct * 128
                first = True
                while c0 < PK:
                    w = min(512, PK - c0)
                    ps = psum.tile([128, 512], F32, tag="ps", name="ps_g")
                    for s in range(N_JT // 2):
                        nc.tensor.matmul(
                            ps[:, 0:w],
                            ht_f8[:, 2 * s:2 * s + 2,
                                  ct * 128:(ct + 1) * 128],
                            ht_f8[:, 2 * s:2 * s + 2, c0:c0 + w],
                            start=(s == 0), stop=(s == N_JT // 2 - 1),
                            perf_mode=mybir.MatmulPerfMode.DoubleRow)
                    g_scr = gscr_pool.tile([128, 512], F32, tag="gscr",
                                           name="g_scr")
                    nc.scalar.activation(
                        out=g_scr[:, 0:w], in_=ps[:, 0:w], func=AF.Square,
                        accum_out=loss_sb[:, ci:ci + 1])
                    if first:
                        # diagonal 128x128 block sits at chunk offset 0
                        d_scr = gscr_pool.tile([128, 128], F32, tag="dscr",
                                               name="d_scr")
                        nc.vector.tensor_mul(d_scr, ps[:, 0:128], eye_sb)
                        nc.vector.reduce_sum(
                            loss_sb[:, 20 + ct:20 + ct + 1], d_scr,
                            axis=mybir.AxisListType.X)
                        nc.vector.reduce_sum(
                            loss_sb[:, 12 + ct:12 + ct + 1],
                            g_scr[:, 0:128], axis=mybir.AxisListType.X)
                        first = False
                    ci += 1
                    c0 += w

            # ---- Phase 4: out = Q0 @ H + bq ----
            for jb in range(N_JB):
                for kt in range(N_CT):
                    ps = psum.tile([128, 512], F32, tag="ps", name="ps_o")
                    for it in range(N_PT):
                        nc.tensor.matmul(
                            ps,
                            qt_sb[:, it, kt * 128:(kt + 1) * 128],
                            hn_sb[:, it, jb * 512:(jb + 1) * 512],
                            start=(it == 0), stop=(it == N_PT - 1))
                    o_t = ostage.tile([128, 512], F32, tag="ot", name="o_t")
                    nc.scalar.activation(out=o_t, in_=ps, func=AF.Identity,
                                         bias=bq_sb[:, kt:kt + 1])
                    nc.sync.dma_start(
                        out=out_d[kt * 128:(kt + 1) * 128,
                                  jb * 512:(jb + 1) * 512],
                        in_=o_t)
            # emitted last so the xbar-mode serialization chain doesn't
            # stall the dma transposes behind this Gram-dependent copy
            nc.gpsimd.dma_start(out=loss_d, in_=loss_sb)


def get_nc():
    if "nc" not in _CACHE:
        _CACHE["nc"] = _build_nc()
    return _CACHE["nc"]


def make_in_maps(Xq, Xk, Wq, bq, Wk, bk, Wv, bv):
    bf = ml_dtypes.bfloat16
    wqt = np.ascontiguousarray(np.transpose(np.asarray(Wq), (0, 2, 1))) \
        .reshape(C, CG).astype(bf)
    wkt = np.ascontiguousarray(np.transpose(np.asarray(Wk), (0, 2, 1))) \
        .reshape(C, CG).astype(bf)
    wvt = np.ascontiguousarray(np.transpose(np.asarray(Wv), (0, 2, 1))) \
        .reshape(C, CG).astype(bf)
    bq32 = np.asarray(bq, np.float32)
    bk32 = np.asarray(bk, np.float32)
    bv32 = np.asarray(bv, np.float32)
    in_maps = []
    for b in range(B):
        in_maps.append({
            "xq": np.ascontiguousarray(np.asarray(Xq[b])).astype(bf),
            "xk": np.ascontiguousarray(np.asarray(Xk[b])).astype(bf),
            "wqt": wqt, "wkt": wkt, "wvt": wvt,
            "bq": bq32, "bk": bk32, "bv": bv32,
        })
    return in_maps


def postprocess(results):
    out = np.stack([np.asarray(r["out"], np.float32) for r in results])
    H = np.stack([np.asarray(r["ht"]).astype(np.float32).T
                  for r in results])[:, None]
    total = 0.0
    s2 = float(F8S) ** 2  # Gram psum holds F8S^2 * mid
    for r in results:
        lp = np.asarray(r["loss"], np.float64)
        # full-matrix sum(mid^2) = 2*sum(computed stair) - sum(diag blocks)
        sq_full = (2.0 * lp[:, :12].sum() - lp[:, 12:20].sum()) / (s2 * s2)
        total += sq_full - 2.0 * lp[:, 20:28].sum() / s2 + float(PK)
    sparse_loss = np.float32(total / (B * PK * PK))
    return out, sparse_loss, H


def kernel(Xq, Xk, Wq, bq, Wk, bk, Wv, bv):
    nc = get_nc()
    in_maps = make_in_maps(Xq, Xk, Wq, bq, Wk, bk, Wv, bv)
    trace = bool(int(os.environ.get("KERNEL_TRACE", "0")))
    res = run_bass_kernel_spmd(nc, in_maps, core_ids=list(range(B)),
                               trace=trace)
    if trace:
        _CACHE["last_result"] = res
    return postprocess(res.results)


# revision 20
# speedup vs baseline: 1.0491x; 1.0491x over previous
"""Trainium2 Bass kernel for nn_CrossTransFormer (grouped-proj cross attention
with Gram-matrix sparsity loss).

Reference computation (per batch b; C=1024, G=4 groups, Pk=1024, Pq=4096):
    Q = blockdiag(Wq) @ Xk + bq          [C, Pk]
    K = blockdiag(Wk) @ Xk + bk          [C, Pk]
    V = blockdiag(Wv) @ Xq + bv          [C, Pq]
    Hraw[i,j] = sum_c K[c,i] V[c,j] * SCALE       [Pk, Pq]
    H = softmax(Hraw, axis=i)
    mid = H @ H.T                        [Pk, Pk]
    sparse_loss = mean((mid - I)^2)      (over all batches)
    out = Q @ H + 0                      [C, Pq]   (out[k,j] = sum_i Q[k,i] H[i,j])

Sharding: data-parallel over batch, one batch per NeuronCore (8 cores).

Per-core dataflow (all matmuls bf16 with fp32 PSUM accumulation):
  - K  [C part, Pk]        k[co,p] = Wk-proj(Xk) + bk   (bias per-partition)
  - QT [Pk part, C]        qt[p,co] = Q0^T (NO bias: out = Q0@H + bq since
                           softmax columns sum to 1 -> bq added at out evac)
  - V  [C part, Pq-block]  streamed per 512-col block of Pq
  - HT [Pq part, Pk]       ht[j,i] = exp(SCALE * sum_c V[c,j]K[c,i]) / S[j]
                           (exp via ScalarE with accum_out = row sum S[j];
                           softmax over i = free axis -> per-partition scalar)
  - ht -> HBM (bf16) is the H output (host transposes + casts)
  - Gram mid = HT^T-contraction over j (= partitions) directly from ht tiles;
    loss partials: sum(mid^2) per tile via ACT Square accum_out, diagonal via
    tensor_tensor_reduce against identity; host: sum(sq) - 2*sum(diag) + Pk.
  - out: H (untransposed) recovered via dma_start_transpose from the ht HBM
    tensor; out[k,j] accumulated over Pk tiles; bias bq per-partition at evac.
"""

import os
from contextlib import ExitStack

import ml_dtypes
import numpy as np

import concourse.bass as bass
import concourse.mybir as mybir
import concourse.tile as tile
from concourse import bacc
from concourse.bass_utils import run_bass_kernel_spmd
from concourse.masks import make_identity

B = 8
C = 1024
G = 4
CG = C // G  # 256
PK = 1024
PQ = 4096
SCALE = float(1.0 / np.float32(np.sqrt(np.float32(1024.0))))

BF16 = mybir.dt.bfloat16
F32 = mybir.dt.float32
F8 = mybir.dt.float8e4
# fp8 pre-scale for the Gram operand: softmax weights (~1e-3) sit below
# fp8e4m3's subnormal range; x64 recenters them (validated: loss rel err
# 4e-5 vs 1e-2 unscaled)
F8S = 64.0
AF = mybir.ActivationFunctionType
ALU = mybir.AluOpType

N_CT = C // 128   # 8 channel tiles
N_PT = PK // 128  # 8 pk tiles
N_JT = PQ // 128  # 32 pq tiles
N_JB = PQ // 512  # 8 pq blocks
N_IB = PK // 512  # 2 pk blocks

_CACHE: dict = {}


def _build_nc():
    nc = bacc.Bacc("TRN2", target_bir_lowering=False, debug=False)

    # ---- DRAM I/O (per-core slices) ----
    xq_d = nc.dram_tensor("xq", [C, PQ], BF16, kind="ExternalInput").ap()
    xk_d = nc.dram_tensor("xk", [C, PK], BF16, kind="ExternalInput").ap()
    wqt_d = nc.dram_tensor("wqt", [C, CG], BF16, kind="ExternalInput").ap()
    wkt_d = nc.dram_tensor("wkt", [C, CG], BF16, kind="ExternalInput").ap()
    wvt_d = nc.dram_tensor("wvt", [C, CG], BF16, kind="ExternalInput").ap()
    bq_d = nc.dram_tensor("bq", [C], F32, kind="ExternalInput").ap()
    bk_d = nc.dram_tensor("bk", [C], F32, kind="ExternalInput").ap()
    bv_d = nc.dram_tensor("bv", [C], F32, kind="ExternalInput").ap()

    out_d = nc.dram_tensor("out", [C, PQ], F32, kind="ExternalOutput").ap()
    ht_d = nc.dram_tensor("ht", [PQ, PK], BF16, kind="ExternalOutput").ap()
    # loss partials (Gram computed on upper stair only, mid is symmetric):
    #  cols 0:12  = sum(mid^2) per computed chunk
    #  cols 12:20 = sum(mid^2) over the 8 diagonal 128x128 blocks
    #  cols 20:28 = sum(diag(mid)) per pk-tile
    loss_d = nc.dram_tensor("loss", [128, 28], F32, kind="ExternalOutput").ap()

    with TileCtx(nc) as (tc, ctx):
        _emit(nc, tc, ctx, xq_d, xk_d, wqt_d, wkt_d, wvt_d, bq_d, bk_d, bv_d,
              out_d, ht_d, loss_d)
    nc.compile()
    return nc


class TileCtx:
    def __init__(self, nc):
        self.nc = nc

    def __enter__(self):
        self.ctx = ExitStack()
        self.tc = self.ctx.enter_context(tile.TileContext(self.nc))
        return self.tc, self.ctx

    def __exit__(self, *exc):
        return self.ctx.__exit__(*exc)


def _emit(nc, tc, ctx, xq_d, xk_d, wqt_d, wkt_d, wvt_d, bq_d, bk_d, bv_d,
          out_d, ht_d, loss_d):
    # ---- persistent pools ----
    consts = ctx.enter_context(tc.tile_pool(name="consts", bufs=1))
    qt_pool = ctx.enter_context(tc.tile_pool(name="qt", bufs=1))
    psum = ctx.enter_context(tc.tile_pool(name="psum", bufs=8, space="PSUM"))
    acc_pool = ctx.enter_context(tc.tile_pool(name="accs", bufs=1))

    # ---- constants: all on the sync ring, ordered so the first K-proj
    # matmul only waits for wkt + the first two xk tiles ----
    wqt_sb = consts.tile([128, N_CT, CG], BF16, name="wqt_sb")
    wkt_sb = consts.tile([128, N_CT, CG], BF16, name="wkt_sb")
    wvt_sb = consts.tile([128, N_CT, CG], BF16, name="wvt_sb")
    bq_sb = consts.tile([128, N_CT], F32, name="bq_sb")
    bk_sb = consts.tile([128, N_CT], F32, name="bk_sb")
    bv_sb = consts.tile([128, N_CT], F32, name="bv_sb")
    nc.sync.dma_start(out=wkt_sb, in_=wkt_d.rearrange("(t p) c -> p t c", p=128))
    eye_sb = consts.tile([128, 128], F32, name="eye_sb")
    make_identity(nc, eye_sb)

    loss_sb = acc_pool.tile([128, 28], F32, name="loss_sb")

    qt_sb = qt_pool.tile([128, N_PT, C], BF16, name="qt_sb")

    f8_pool = ctx.enter_context(tc.tile_pool(name="f8", bufs=1))
    # Gram operand: H^T normalized, pre-scaled by F8S, cast to fp8e4m3.
    ht_f8 = f8_pool.tile([128, N_JT, PK], F8, name="ht_f8")

    with tc.tile_pool(name="big", bufs=1) as big_pool:
        ht_sb = big_pool.tile([128, N_JT, PK], BF16, name="ht_sb")

        with tc.tile_pool(name="ph12", bufs=1) as ph12, \
             tc.tile_pool(name="stream", bufs=2) as stream, \
             tc.tile_pool(name="evac", bufs=3) as evac, \
             tc.tile_pool(name="small", bufs=6) as small:
            xk_sb = ph12.tile([128, N_CT, PK], BF16, name="xk_sb")
            k_sb = ph12.tile([128, N_CT, PK], BF16, name="k_sb")
            xk_r = xk_d.rearrange("(t p) c -> p t c", p=128)
            nc.sync.dma_start(out=xk_sb[:, 0:2], in_=xk_r[:, 0:2])
            nc.sync.dma_start(out=bk_sb,
                              in_=bk_d.rearrange("(t p) -> p t", p=128))
            nc.sync.dma_start(out=xk_sb[:, 2:4], in_=xk_r[:, 2:4])
            nc.sync.dma_start(out=wqt_sb,
                              in_=wqt_d.rearrange("(t p) c -> p t c", p=128))
            nc.sync.dma_start(out=xk_sb[:, 4:6], in_=xk_r[:, 4:6])
            nc.sync.dma_start(out=xk_sb[:, 6:8], in_=xk_r[:, 6:8])
            nc.sync.dma_start(out=wvt_sb,
                              in_=wvt_d.rearrange("(t p) c -> p t c", p=128))
            nc.sync.dma_start(out=bv_sb,
                              in_=bv_d.rearrange("(t p) -> p t", p=128))
            nc.sync.dma_start(out=bq_sb,
                              in_=bq_d.rearrange("(t p) -> p t", p=128))

            # ---- Phase 1a: K projection -> k_sb [co part, pk] + bias ----
            for ct in range(N_CT):
                g, h = ct // 2, ct % 2
                for pb in range(N_IB):
                    ps = psum.tile([128, 512], F32, tag="ps", name="ps_k")
                    for ci in range(2):
                        nc.tensor.matmul(
                            ps,
                            wkt_sb[:, 2 * g + ci, h * 128:(h + 1) * 128],
                            xk_sb[:, 2 * g + ci, pb * 512:(pb + 1) * 512],
                            start=(ci == 0), stop=(ci == 1))
                    nc.vector.tensor_scalar_add(
                        k_sb[:, ct, pb * 512:(pb + 1) * 512], ps,
                        bk_sb[:, ct:ct + 1])

            # ---- Phase 1b: QT projection -> qt_sb [pk part, co] (no bias) ----
            for pt in range(N_PT):
                for g in range(G):
                    ps = psum.tile([128, 512], F32, tag="ps", name="ps_qt")
                    for ci in range(2):
                        nc.tensor.matmul(
                            ps[:, 0:CG],
                            xk_sb[:, 2 * g + ci, pt * 128:(pt + 1) * 128],
                            wqt_sb[:, 2 * g + ci, :],
                            start=(ci == 0), stop=(ci == 1))
                    nc.vector.tensor_copy(
                        qt_sb[:, pt, g * CG:(g + 1) * CG], ps[:, 0:CG])

            # ---- Phase 2: V projection + HT matmul + softmax, per Pq block --
            for jb in range(N_JB):
                xq_blk = stream.tile([128, N_CT, 512], BF16, tag="xq",
                                     name="xq_blk")
                nc.sync.dma_start(
                    out=xq_blk,
                    in_=xq_d[:, jb * 512:(jb + 1) * 512]
                        .rearrange("(t p) c -> p t c", p=128))
                v_blk = stream.tile([128, N_CT, 512], BF16, tag="v",
                                    name="v_blk")
                for ct in range(N_CT):
                    g, h = ct // 2, ct % 2
                    ps = psum.tile([128, 512], F32, tag="ps", name="ps_v")
                    for ci in range(2):
                        nc.tensor.matmul(
                            ps,
                            wvt_sb[:, 2 * g + ci, h * 128:(h + 1) * 128],
                            xq_blk[:, 2 * g + ci, :],
                            start=(ci == 0), stop=(ci == 1))
                    nc.scalar.activation(
                        out=v_blk[:, ct, :], in_=ps, func=AF.Identity,
                        bias=bv_sb[:, ct:ct + 1])

                for jt4 in range(4):
                    jt = jb * 4 + jt4
                    sacc = small.tile([128, 2], F32, tag="sacc", name="sacc")
                    for ib in range(N_IB):
                        ps = psum.tile([128, 512], F32, tag="ps", name="ps_h")
                        for ct in range(N_CT):
                            nc.tensor.matmul(
                                ps,
                                v_blk[:, ct, jt4 * 128:(jt4 + 1) * 128],
                                k_sb[:, ct, ib * 512:(ib + 1) * 512],
                                start=(ct == 0), stop=(ct == N_CT - 1))
                        nc.scalar.activation(
                            out=ht_sb[:, jt, ib * 512:(ib + 1) * 512],
                            in_=ps, func=AF.Exp, scale=SCALE,
                            accum_out=sacc[:, ib:ib + 1])
                    s_t = small.tile([128, 1], F32, tag="s", name="s_t")
                    nc.vector.tensor_add(s_t, sacc[:, 0:1], sacc[:, 1:2])
                    rs_t = small.tile([128, 1], F32, tag="rs", name="rs_t")
                    nc.vector.reciprocal(rs_t, s_t)
                    nc.vector.tensor_scalar_mul(
                        ht_sb[:, jt, :], ht_sb[:, jt, :], rs_t)
                    nc.sync.dma_start(out=ht_d[jt * 128:(jt + 1) * 128, :],
                                      in_=ht_sb[:, jt, :])
                    nc.vector.tensor_scalar_mul(
                        ht_f8[:, jt, :], ht_sb[:, jt, :], F8S)

        # ---- Phases 3+4 (ph12/stream pools closed; hn coexists with ht) --
        with tc.tile_pool(name="hn", bufs=1) as hn_pool, \
             tc.tile_pool(name="gscr", bufs=3) as gscr_pool, \
             tc.tile_pool(name="ostage", bufs=3) as ostage:
            # transpose-load H from the ht HBM tensor; emitted before the
            # Gram matmuls so the DMAs overlap Gram compute on the PE.
            hn_sb = hn_pool.tile([128, N_PT, PQ], BF16, name="hn_sb")
            for it in range(N_PT):
                nc.sync.dma_start_transpose(
                    out=hn_sb[:, it, :], in_=ht_d[:, it * 128:(it + 1) * 128])

            # ---- Phase 3: Gram upper-stair chunks + loss partials ----
            # mid is symmetric: for pk-tile ct compute columns [ct*128, PK)
            # in chunks of <=512.  Host reconstructs the full-matrix sums.
            ci = 0
            for ct in range(N_PT):
                c0 = ct * 128
                first = True
                while c0 < PK:
                    w = min(512, PK - c0)
                    ps = psum.tile([128, 512], F32, tag="ps", name="ps_g")
                    for s in range(N_JT // 2):
                        nc.tensor.matmul(
                            ps[:, 0:w],
                            ht_f8[:, 2 * s:2 * s + 2,
                                  ct * 128:(ct + 1) * 128],
                            ht_f8[:, 2 * s:2 * s + 2, c0:c0 + w],
                            start=(s == 0), stop=(s == N_JT // 2 - 1),
                            perf_mode=mybir.MatmulPerfMode.DoubleRow)
                    g_scr = gscr_pool.tile([128, 512], F32, tag="gscr",
                                           name="g_scr")
                    nc.scalar.activation(
                        out=g_scr[:, 0:w], in_=ps[:, 0:w], func=AF.Square,
                        accum_out=loss_sb[:, ci:ci + 1])
                    if first:
                        # diagonal 128x128 block sits at chunk offset 0
                        d_scr = gscr_pool.tile([128, 128], F32, tag="dscr",
                                               name="d_scr")
                        nc.vector.tensor_mul(d_scr, ps[:, 0:128], eye_sb)
                        nc.vector.reduce_sum(
                            loss_sb[:, 20 + ct:20 + ct + 1], d_scr,
                            axis=mybir.AxisListType.X)
                        nc.vector.reduce_sum(
                            loss_sb[:, 12 + ct:12 + ct + 1],
                            g_scr[:, 0:128], axis=mybir.AxisListType.X)
                        first = False
                    ci += 1
                    c0 += w

            # ---- Phase 4: out = Q0 @ H + bq ----
            for jb in range(N_JB):
                for kt in range(N_CT):
                    ps = psum.tile([128, 512], F32, tag="ps", name="ps_o")
                    for it in range(N_PT):
                        nc.tensor.matmul(
                            ps,
                            qt_sb[:, it, kt * 128:(kt + 1) * 128],
                            hn_sb[:, it, jb * 512:(jb + 1) * 512],
                            start=(it == 0), stop=(it == N_PT - 1))
                    o_t = ostage.tile([128, 512], F32, tag="ot", name="o_t")
                    nc.scalar.activation(out=o_t, in_=ps, func=AF.Identity,
                                         bias=bq_sb[:, kt:kt + 1])
                    nc.sync.dma_start(
                        out=out_d[kt * 128:(kt + 1) * 128,
                                  jb * 512:(jb + 1) * 512],
                        in_=o_t)
            # emitted last on the sync ring (strict FIFO) so the xbar-mode
            # serialization chain can't stall the dma transposes behind
            # this Gram-dependent copy
            nc.sync.dma_start(out=loss_d, in_=loss_sb)


def get_nc():
    if "nc" not in _CACHE:
        _CACHE["nc"] = _build_nc()
    return _CACHE["nc"]


def make_in_maps(Xq, Xk, Wq, bq, Wk, bk, Wv, bv):
    bf = ml_dtypes.bfloat16
    wqt = np.ascontiguousarray(np.transpose(np.asarray(Wq), (0, 2, 1))) \
        .reshape(C, CG).astype(bf)
    wkt = np.ascontiguousarray(np.transpose(np.asarray(Wk), (0, 2, 1))) \
        .reshape(C, CG).astype(bf)
    wvt = np.ascontiguousarray(np.transpose(np.asarray(Wv), (0, 2, 1))) \
        .reshape(C, CG).astype(bf)
    bq32 = np.asarray(bq, np.float32)
    bk32 = np.asarray(bk, np.float32)
    bv32 = np.asarray(bv, np.float32)
    in_maps = []
    for b in range(B):
        in_maps.append({
            "xq": np.ascontiguousarray(np.asarray(Xq[b])).astype(bf),
            "xk": np.ascontiguousarray(np.asarray(Xk[b])).astype(bf),
            "wqt": wqt, "wkt": wkt, "wvt": wvt,
            "bq": bq32, "bk": bk32, "bv": bv32,
        })
    return in_maps


def postprocess(results):
    out = np.stack([np.asarray(r["out"], np.float32) for r in results])
    H = np.stack([np.asarray(r["ht"]).astype(np.float32).T
                  for r in results])[:, None]
    total = 0.0
    s2 = float(F8S) ** 2  # Gram psum holds F8S^2 * mid
    for r in results:
        lp = np.asarray(r["loss"], np.float64)
        # full-matrix sum(mid^2) = 2*sum(computed stair) - sum(diag blocks)
        sq_full = (2.0 * lp[:, :12].sum() - lp[:, 12:20].sum()) / (s2 * s2)
        total += sq_full - 2.0 * lp[:, 20:28].sum() / s2 + float(PK)
    sparse_loss = np.float32(total / (B * PK * PK))
    return out, sparse_loss, H


def kernel(Xq, Xk, Wq, bq, Wk, bk, Wv, bv):
    nc = get_nc()
    in_maps = make_in_maps(Xq, Xk, Wq, bq, Wk, bk, Wv, bv)
    trace = bool(int(os.environ.get("KERNEL_TRACE", "0")))
    res = run_bass_kernel_spmd(nc, in_maps, core_ids=list(range(B)),
                               trace=trace)
    if trace:
        _CACHE["last_result"] = res
    return postprocess(res.results)


# revision 25
# speedup vs baseline: 1.1424x; 1.0889x over previous
"""Trainium2 Bass kernel for nn_CrossTransFormer (grouped-proj cross attention
with Gram-matrix sparsity loss).

Reference computation (per batch b; C=1024, G=4 groups, Pk=1024, Pq=4096):
    Q = blockdiag(Wq) @ Xk + bq          [C, Pk]
    K = blockdiag(Wk) @ Xk + bk          [C, Pk]
    V = blockdiag(Wv) @ Xq + bv          [C, Pq]
    Hraw[i,j] = sum_c K[c,i] V[c,j] * SCALE       [Pk, Pq]
    H = softmax(Hraw, axis=i)
    mid = H @ H.T                        [Pk, Pk]
    sparse_loss = mean((mid - I)^2)      (over all batches)
    out = Q @ H + 0                      [C, Pq]   (out[k,j] = sum_i Q[k,i] H[i,j])

Sharding: data-parallel over batch, one batch per NeuronCore (8 cores).

Per-core dataflow (all matmuls bf16 with fp32 PSUM accumulation):
  - K  [C part, Pk]        k[co,p] = Wk-proj(Xk) + bk   (bias per-partition)
  - QT [Pk part, C]        qt[p,co] = Q0^T (NO bias: out = Q0@H + bq since
                           softmax columns sum to 1 -> bq added at out evac)
  - V  [C part, Pq-block]  streamed per 512-col block of Pq
  - HT [Pq part, Pk]       ht[j,i] = exp(SCALE * sum_c V[c,j]K[c,i]) / S[j]
                           (exp via ScalarE with accum_out = row sum S[j];
                           softmax over i = free axis -> per-partition scalar)
  - ht -> HBM (bf16) is the H output (host transposes + casts)
  - Gram mid = HT^T-contraction over j (= partitions) directly from ht tiles;
    loss partials: sum(mid^2) per tile via ACT Square accum_out, diagonal via
    tensor_tensor_reduce against identity; host: sum(sq) - 2*sum(diag) + Pk.
  - out: H (untransposed) recovered via dma_start_transpose from the ht HBM
    tensor; out[k,j] accumulated over Pk tiles; bias bq per-partition at evac.
"""

import os
from contextlib import ExitStack

import ml_dtypes
import numpy as np

import concourse.bass as bass
import concourse.mybir as mybir
import concourse.tile as tile
from concourse import bacc
from concourse.bass_utils import run_bass_kernel_spmd
from concourse.masks import make_identity

B = 8
C = 1024
G = 4
CG = C // G  # 256
PK = 1024
PQ = 4096
SCALE = float(1.0 / np.float32(np.sqrt(np.float32(1024.0))))

BF16 = mybir.dt.bfloat16
F32 = mybir.dt.float32
F8 = mybir.dt.float8e4
# fp8 pre-scale for the Gram operand: softmax weights (~1e-3) sit below
# fp8e4m3's subnormal range; x64 recenters them (validated: loss rel err
# 4e-5 vs 1e-2 unscaled)
F8S = 64.0
AF = mybir.ActivationFunctionType
ALU = mybir.AluOpType

N_CT = C // 128   # 8 channel tiles
N_PT = PK // 128  # 8 pk tiles
N_JT = PQ // 128  # 32 pq tiles
N_JB = PQ // 512  # 8 pq blocks
N_IB = PK // 512  # 2 pk blocks

_CACHE: dict = {}


def _build_nc():
    nc = bacc.Bacc("TRN2", target_bir_lowering=False, debug=False)

    # ---- DRAM I/O (per-core slices) ----
    # K and V are algebraically folded (host-side):
    #   Hraw^T = Xq^T (WW Xk + bvk) + 1 kbv^T,  WW = Wv^T Wk (per group),
    #   bvk = Wv^T bk, kbv = (Wk^T bv)^T Xk + bv.bk
    xq_d = nc.dram_tensor("xq", [C, PQ], BF16, kind="ExternalInput").ap()
    xk_d = nc.dram_tensor("xk", [C, PK], BF16, kind="ExternalInput").ap()
    wqt_d = nc.dram_tensor("wqt", [C, CG], BF16, kind="ExternalInput").ap()
    wwt_d = nc.dram_tensor("wwt", [C, CG], BF16, kind="ExternalInput").ap()
    bq_d = nc.dram_tensor("bq", [C], F32, kind="ExternalInput").ap()
    bvk_d = nc.dram_tensor("bvk", [C], F32, kind="ExternalInput").ap()
    r_d = nc.dram_tensor("rv", [C], BF16, kind="ExternalInput").ap()
    s0_d = nc.dram_tensor("s0", [1], F32, kind="ExternalInput").ap()

    out_d = nc.dram_tensor("out", [C, PQ], F32, kind="ExternalOutput").ap()
    ht_d = nc.dram_tensor("ht", [PQ, PK], BF16, kind="ExternalOutput").ap()
    # loss partials (Gram computed on upper stair only, mid is symmetric):
    #  cols 0:12  = sum(mid^2) per computed chunk
    #  cols 12:20 = sum(mid^2) over the 8 diagonal 128x128 blocks
    #  cols 20:28 = sum(diag(mid)) per pk-tile
    loss_d = nc.dram_tensor("loss", [128, 28], F32, kind="ExternalOutput").ap()

    with TileCtx(nc) as (tc, ctx):
        _emit(nc, tc, ctx, xq_d, xk_d, wqt_d, wwt_d, bq_d, bvk_d, r_d, s0_d,
              out_d, ht_d, loss_d)
    nc.compile()
    return nc


class TileCtx:
    def __init__(self, nc):
        self.nc = nc

    def __enter__(self):
        self.ctx = ExitStack()
        self.tc = self.ctx.enter_context(tile.TileContext(self.nc))
        return self.tc, self.ctx

    def __exit__(self, *exc):
        return self.ctx.__exit__(*exc)


def _emit(nc, tc, ctx, xq_d, xk_d, wqt_d, wwt_d, bq_d, bvk_d, r_d, s0_d,
          out_d, ht_d, loss_d):
    # ---- persistent pools ----
    consts = ctx.enter_context(tc.tile_pool(name="consts", bufs=1))
    qt_pool = ctx.enter_context(tc.tile_pool(name="qt", bufs=1))
    psum = ctx.enter_context(tc.tile_pool(name="psum", bufs=8, space="PSUM"))
    acc_pool = ctx.enter_context(tc.tile_pool(name="accs", bufs=1))
    dram = ctx.enter_context(tc.tile_pool(name="dram", bufs=1, space="DRAM"))

    # ---- constants: all on the sync ring, ordered so the first KWvT-proj
    # matmul only waits for wwt + the first two xk tiles ----
    wqt_sb = consts.tile([128, N_CT, CG], BF16, name="wqt_sb")
    wwt_sb = consts.tile([128, N_CT, CG], BF16, name="wwt_sb")
    bq_sb = consts.tile([128, N_CT], F32, name="bq_sb")
    bvk_sb = consts.tile([128, N_CT], F32, name="bvk_sb")
    r_sb = consts.tile([128, N_CT], BF16, name="r_sb")
    s0_sb = consts.tile([128, 1], F32, name="s0_sb")
    kbv_bc = consts.tile([128, PK], F32, name="kbv_bc")
    nc.sync.dma_start(out=wwt_sb, in_=wwt_d.rearrange("(t p) c -> p t c", p=128))
    eye_sb = consts.tile([128, 128], F32, name="eye_sb")
    make_identity(nc, eye_sb)

    loss_sb = acc_pool.tile([128, 28], F32, name="loss_sb")

    qt_sb = qt_pool.tile([128, N_PT, C], BF16, name="qt_sb")

    f8_pool = ctx.enter_context(tc.tile_pool(name="f8", bufs=1))
    # Gram operand: H^T normalized, pre-scaled by F8S, cast to fp8e4m3.
    ht_f8 = f8_pool.tile([128, N_JT, PK], F8, name="ht_f8")

    with tc.tile_pool(name="big", bufs=1) as big_pool:
        ht_sb = big_pool.tile([128, N_JT, PK], BF16, name="ht_sb")

        with tc.tile_pool(name="ph12", bufs=1) as ph12, \
             tc.tile_pool(name="stream", bufs=3) as stream, \
             tc.tile_pool(name="small", bufs=6) as small:
            xk_sb = ph12.tile([128, N_CT, PK], BF16, name="xk_sb")
            kwv_sb = ph12.tile([128, N_CT, PK], BF16, name="kwv_sb")
            xk_r = xk_d.rearrange("(t p) c -> p t c", p=128)
            nc.sync.dma_start(out=xk_sb[:, 0:2], in_=xk_r[:, 0:2])
            nc.sync.dma_start(out=bvk_sb,
                              in_=bvk_d.rearrange("(t p) -> p t", p=128))
            nc.sync.dma_start(out=xk_sb[:, 2:4], in_=xk_r[:, 2:4])
            nc.sync.dma_start(out=wqt_sb,
                              in_=wqt_d.rearrange("(t p) c -> p t c", p=128))
            nc.sync.dma_start(out=xk_sb[:, 4:6], in_=xk_r[:, 4:6])
            nc.sync.dma_start(out=xk_sb[:, 6:8], in_=xk_r[:, 6:8])
            nc.sync.dma_start(out=r_sb,
                              in_=r_d.rearrange("(t p) -> p t", p=128))
            nc.sync.dma_start(out=s0_sb[0:1, :],
                              in_=s0_d.rearrange("(a b) -> a b", a=1))
            nc.sync.dma_start(out=bq_sb,
                              in_=bq_d.rearrange("(t p) -> p t", p=128))

            # ---- Phase 1a: KWvT projection -> kwv_sb [c part, pk] + bvk --
            for ct in range(N_CT):
                g, h = ct // 2, ct % 2
                for pb in range(N_IB):
                    ps = psum.tile([128, 512], F32, tag="ps", name="ps_k")
                    for ci in range(2):
                        nc.tensor.matmul(
                            ps,
                            wwt_sb[:, 2 * g + ci, h * 128:(h + 1) * 128],
                            xk_sb[:, 2 * g + ci, pb * 512:(pb + 1) * 512],
                            start=(ci == 0), stop=(ci == 1))
                    nc.vector.tensor_scalar_add(
                        kwv_sb[:, ct, pb * 512:(pb + 1) * 512], ps,
                        bvk_sb[:, ct:ct + 1])

            # ---- Phase 1b: QT projection -> qt_sb [pk part, co] (no bias) ----
            for pt in range(N_PT):
                for g in range(G):
                    ps = psum.tile([128, 512], F32, tag="ps", name="ps_qt")
                    for ci in range(2):
                        nc.tensor.matmul(
                            ps[:, 0:CG],
                            xk_sb[:, 2 * g + ci, pt * 128:(pt + 1) * 128],
                            wqt_sb[:, 2 * g + ci, :],
                            start=(ci == 0), stop=(ci == 1))
                    nc.vector.tensor_copy(
                        qt_sb[:, pt, g * CG:(g + 1) * CG], ps[:, 0:CG])

            # ---- Phase 1c: kbv row = r^T Xk + s0, broadcast to all
            # partitions via an HBM round trip ----
            kbv_dram = dram.tile([PK], F32, name="kbv_dram")
            kbv_row = small.tile([128, PK], F32, tag="kbvrow", bufs=1,
                                 name="kbv_row")
            for ib in range(N_IB):
                ps = psum.tile([128, 512], F32, tag="ps", name="ps_r")
                for ct in range(N_CT):
                    nc.tensor.matmul(
                        ps[0:1, :],
                        r_sb[:, ct:ct + 1],
                        xk_sb[:, ct, ib * 512:(ib + 1) * 512],
                        start=(ct == 0), stop=(ct == N_CT - 1))
                nc.scalar.activation(
                    out=kbv_row[0:1, ib * 512:(ib + 1) * 512],
                    in_=ps[0:1, :], func=AF.Identity, bias=s0_sb[0:1, :])
            nc.sync.dma_start(
                out=kbv_dram.rearrange("(a b) -> a b", a=1),
                in_=kbv_row[0:1, :])
            kbv_bcast_ap = bass.AP(
                tensor=kbv_dram.tensor, offset=kbv_dram.offset,
                ap=[[0, 128], [1, PK]])
            nc.gpsimd.dma_start(out=kbv_bc, in_=kbv_bcast_ap)

            # ---- Phase 2: HT matmul + softmax, per Pq block ----
            for jb in range(N_JB):
                xq_blk = stream.tile([128, N_CT, 512], BF16, tag="xq",
                                     name="xq_blk")
                nc.sync.dma_start(
                    out=xq_blk,
                    in_=xq_d[:, jb * 512:(jb + 1) * 512]
                        .rearrange("(t p) c -> p t c", p=128))

                for jt4 in range(4):
                    jt = jb * 4 + jt4
                    sacc = small.tile([128, 2], F32, tag="sacc", name="sacc")
                    for ib in range(N_IB):
                        ps = psum.tile([128, 512], F32, tag="ps", name="ps_h")
                        for ct in range(N_CT):
                            nc.tensor.matmul(
                                ps,
                                xq_blk[:, ct, jt4 * 128:(jt4 + 1) * 128],
                                kwv_sb[:, ct, ib * 512:(ib + 1) * 512],
                                start=(ct == 0), stop=(ct == N_CT - 1))
                        nc.vector.tensor_add(
                            ps, ps, kbv_bc[:, ib * 512:(ib + 1) * 512])
                        nc.scalar.activation(
                            out=ht_sb[:, jt, ib * 512:(ib + 1) * 512],
                            in_=ps, func=AF.Exp, scale=SCALE,
                            accum_out=sacc[:, ib:ib + 1])
                    s_t = small.tile([128, 1], F32, tag="s", name="s_t")
                    nc.vector.tensor_add(s_t, sacc[:, 0:1], sacc[:, 1:2])
                    rs_t = small.tile([128, 1], F32, tag="rs", name="rs_t")
                    nc.vector.reciprocal(rs_t, s_t)
                    nc.vector.tensor_scalar_mul(
                        ht_sb[:, jt, :], ht_sb[:, jt, :], rs_t)
                    nc.sync.dma_start(out=ht_d[jt * 128:(jt + 1) * 128, :],
                                      in_=ht_sb[:, jt, :])
                    nc.vector.tensor_scalar_mul(
                        ht_f8[:, jt, :], ht_sb[:, jt, :], F8S)

        # ---- Phases 3+4 (ph12/stream pools closed; hn coexists with ht) --
        with tc.tile_pool(name="hn", bufs=1) as hn_pool, \
             tc.tile_pool(name="gscr", bufs=3) as gscr_pool, \
             tc.tile_pool(name="ostage", bufs=3) as ostage:
            # transpose-load H from the ht HBM tensor; emitted before the
            # Gram matmuls so the DMAs overlap Gram compute on the PE.
            hn_sb = hn_pool.tile([128, N_PT, PQ], BF16, name="hn_sb")
            for it in range(N_PT):
                nc.sync.dma_start_transpose(
                    out=hn_sb[:, it, :], in_=ht_d[:, it * 128:(it + 1) * 128])

            # ---- Phase 3: Gram upper-stair chunks + loss partials ----
            # mid is symmetric: for pk-tile ct compute columns [ct*128, PK)
            # in chunks of <=512.  Host reconstructs the full-matrix sums.
            ci = 0
            for ct in range(N_PT):
                c0 = ct * 128
                first = True
                while c0 < PK:
                    w = min(512, PK - c0)
                    ps = psum.tile([128, 512], F32, tag="ps", name="ps_g")
                    for s in range(N_JT // 2):
                        nc.tensor.matmul(
                            ps[:, 0:w],
                            ht_f8[:, 2 * s:2 * s + 2,
                                  ct * 128:(ct + 1) * 128],
                            ht_f8[:, 2 * s:2 * s + 2, c0:c0 + w],
                            start=(s == 0), stop=(s == N_JT // 2 - 1),
                            perf_mode=mybir.MatmulPerfMode.DoubleRow)
                    g_scr = gscr_pool.tile([128, 512], F32, tag="gscr",
                                           name="g_scr")
                    nc.scalar.activation(
                        out=g_scr[:, 0:w], in_=ps[:, 0:w], func=AF.Square,
                        accum_out=loss_sb[:, ci:ci + 1])
                    if first:
                        # diagonal 128x128 block sits at chunk offset 0
                        d_scr = gscr_pool.tile([128, 128], F32, tag="dscr",
                                               name="d_scr")
                        nc.vector.tensor_mul(d_scr, ps[:, 0:128], eye_sb)
                        nc.vector.reduce_sum(
                            loss_sb[:, 20 + ct:20 + ct + 1], d_scr,
                            axis=mybir.AxisListType.X)
                        nc.vector.reduce_sum(
                            loss_sb[:, 12 + ct:12 + ct + 1],
                            g_scr[:, 0:128], axis=mybir.AxisListType.X)
                        first = False
                    ci += 1
                    c0 += w

            # ---- Phase 4: out = Q0 @ H + bq ----
            for jb in range(N_JB):
                for kt in range(N_CT):
                    ps = psum.tile([128, 512], F32, tag="ps", name="ps_o")
                    for it in range(N_PT):
                        nc.tensor.matmul(
                            ps,
                            qt_sb[:, it, kt * 128:(kt + 1) * 128],
                            hn_sb[:, it, jb * 512:(jb + 1) * 512],
                            start=(it == 0), stop=(it == N_PT - 1))
                    o_t = ostage.tile([128, 512], F32, tag="ot", name="o_t")
                    nc.scalar.activation(out=o_t, in_=ps, func=AF.Identity,
                                         bias=bq_sb[:, kt:kt + 1])
                    nc.sync.dma_start(
                        out=out_d[kt * 128:(kt + 1) * 128,
                                  jb * 512:(jb + 1) * 512],
                        in_=o_t)
            # emitted last on the sync ring (strict FIFO) so the xbar-mode
            # serialization chain can't stall the dma transposes behind
            # this Gram-dependent copy
            nc.sync.dma_start(out=loss_d, in_=loss_sb)


def get_nc():
    if "nc" not in _CACHE:
        _CACHE["nc"] = _build_nc()
    return _CACHE["nc"]


def make_in_maps(Xq, Xk, Wq, bq, Wk, bk, Wv, bv):
    bf = ml_dtypes.bfloat16
    Wq = np.asarray(Wq, np.float64)
    Wk = np.asarray(Wk, np.float64)
    Wv = np.asarray(Wv, np.float64)
    bk64 = np.asarray(bk, np.float64).reshape(G, CG)
    bv64 = np.asarray(bv, np.float64).reshape(G, CG)
    wqt = np.ascontiguousarray(np.transpose(Wq, (0, 2, 1))) \
        .reshape(C, CG).astype(bf)
    # WW[g] = Wv[g]^T @ Wk[g]; stationary operand needs WW[g]^T rows
    ww = np.einsum("gcx,gcy->gxy", Wv, Wk)        # [g, x(out), y(in)]
    wwt = np.ascontiguousarray(np.transpose(ww, (0, 2, 1))) \
        .reshape(C, CG).astype(bf)
    bvk = np.einsum("gcx,gc->gx", Wv, bk64).reshape(C).astype(np.float32)
    rv = np.einsum("gcx,gc->gx", Wk, bv64).reshape(C).astype(bf)
    s0 = np.array([np.dot(bv64.ravel(), bk64.ravel())], np.float32)
    bq32 = np.asarray(bq, np.float32)
    in_maps = []
    for b in range(B):
        in_maps.append({
            "xq": np.ascontiguousarray(np.asarray(Xq[b])).astype(bf),
            "xk": np.ascontiguousarray(np.asarray(Xk[b])).astype(bf),
            "wqt": wqt, "wwt": wwt,
            "bq": bq32, "bvk": bvk, "rv": rv, "s0": s0,
        })
    return in_maps


def postprocess(results):
    out = np.stack([np.asarray(r["out"], np.float32) for r in results])
    H = np.stack([np.asarray(r["ht"]).astype(np.float32).T
                  for r in results])[:, None]
    total = 0.0
    s2 = float(F8S) ** 2  # Gram psum holds F8S^2 * mid
    for r in results:
        lp = np.asarray(r["loss"], np.float64)
        # full-matrix sum(mid^2) = 2*sum(computed stair) - sum(diag blocks)
        sq_full = (2.0 * lp[:, :12].sum() - lp[:, 12:20].sum()) / (s2 * s2)
        total += sq_full - 2.0 * lp[:, 20:28].sum() / s2 + float(PK)
    sparse_loss = np.float32(total / (B * PK * PK))
    return out, sparse_loss, H


def kernel(Xq, Xk, Wq, bq, Wk, bk, Wv, bv):
    nc = get_nc()
    in_maps = make_in_maps(Xq, Xk, Wq, bq, Wk, bk, Wv, bv)
    trace = bool(int(os.environ.get("KERNEL_TRACE", "0")))
    res = run_bass_kernel_spmd(nc, in_maps, core_ids=list(range(B)),
                               trace=trace)
    if trace:
        _CACHE["last_result"] = res
    return postprocess(res.results)
